# revision 7
# baseline (speedup 1.0000x reference)
"""ECE-loss kernel for Trainium2 (8 NeuronCores, raw Bass).

Strategy (validated against f64 ground truth on the reference dataset):
  - Histogram over 10 confidence bins from cumulative stats:
      G[k] = #{p > e_k},  R[k] = sum relu(p - e_k)   for edges e_1..e_9
    then counts[b] = G[b]-G[b+1], conf_sum[b] = S[b]-S[b+1] with
    S[k] = R[k] + e_k*G[k].  Only edges 0.1..0.4 need the full data:
    any p > 0.5 is necessarily the row max, so edges 0.5..0.9 are
    computed on the 10x smaller per-row max array.
  - Correctness (argmax == label) uses a host-side bit-pack: for positive
    floats the int32 view is order-preserving, so we send
      y = (bitcast_i32(p) & ~15) | (9 - class)
    and a single segmented int-max per row yields both the (truncated)
    max prob and its argmax; ties break toward the smallest class index,
    matching jnp.argmax.
  - Device reduces everything to per-partition scalars via
    tensor_scalar/activation accum_out (DVE: mask counts; ACT: relu sums);
    host does the final 30-scalar math in f64.

Data parallel over N across the 8 cores; per-core output is [128, 64] f32.
"""

import numpy as np

NCORES = 8
N = 2_000_000
C = 10
NS_CORE = N // NCORES            # 250_000 samples per core
P = 128
GP = 1960                        # samples per partition (padded; 128*1960 = 250_880)
NPAD = P * GP
NT = 4                           # data tiles
GT = GP // NT                    # samples per partition per tile
FDT = GT * C                     # free-dim elements per tile
RES_SLOTS = 64

EDGES = np.arange(0.0, 1.1, 0.1).astype(np.float32)   # same 11 edges as reference
EC0 = 0.05                       # below any real row-max (>=0.1), above pad zeros

# res slot map: per tile t: [t*9 .. t*9+8] = cnt_e1..4, relu_e1..4, stot_t
FB = NT * 9                      # finals base = 36
# finals: FB+0..4 cnt_e5..9 | FB+5..9 relu_e5..9 | FB+10..19 gcorr edges [0.05,e1..e9]

_CACHE = {}


def _build_nc():
    from contextlib import ExitStack
    import concourse.bass as bass
    import concourse.mybir as mybir

    A = mybir.AluOpType
    AF = mybir.ActivationFunctionType
    f32 = mybir.dt.float32
    i32 = mybir.dt.int32

    nc = bass.Bass("TRN2", target_bir_lowering=False, debug=False,
                   num_devices=NCORES)
    y = nc.dram_tensor("y", [P, GP * C], f32, kind="ExternalInput")
    lab = nc.dram_tensor("lab", [P, GP], i32, kind="ExternalInput")
    bias = nc.dram_tensor("bias", [P, 16], f32, kind="ExternalInput")
    res = nc.dram_tensor("res", [P, RES_SLOTS], f32, kind="ExternalOutput")

    with ExitStack() as st:
        yb = [st.enter_context(nc.sbuf_tensor(f"y{i}", [P, FDT], f32)) for i in range(2)]
        labt = st.enter_context(nc.sbuf_tensor("labt", [P, GP], i32))
        biast = st.enter_context(nc.sbuf_tensor("biast", [P, 16], f32))
        rest = st.enter_context(nc.sbuf_tensor("rest", [P, RES_SLOTS], f32))
        pmt = st.enter_context(nc.sbuf_tensor("pmt", [P, GP], i32))
        am = st.enter_context(nc.sbuf_tensor("am", [P, GP], i32))
        pmc = st.enter_context(nc.sbuf_tensor("pmc", [P, GP], i32))
        corr = st.enter_context(nc.sbuf_tensor("corr", [P, GP], i32))
        pc = st.enter_context(nc.sbuf_tensor("pc", [P, GP], i32))
        scrV = st.enter_context(nc.sbuf_tensor("scrV", [P, FDT], f32))
        scrA = st.enter_context(nc.sbuf_tensor("scrA", [P, FDT], f32))
        scrF = st.enter_context(nc.sbuf_tensor("scrF", [P, GP], f32))
        dmac = st.enter_context(nc.semaphore("dmac"))
        dmat0 = st.enter_context(nc.semaphore("dmat0"))
        dmat1 = st.enter_context(nc.semaphore("dmat1"))
        vt = st.enter_context(nc.semaphore("vt"))
        at = st.enter_context(nc.semaphore("at"))
        block = st.enter_context(nc.Block())

        dmat = [dmat0, dmat1]

        @block.sync
        def _(sync):
            sync.dma_start(biast[:], bias[:, :]).then_inc(dmac, 16)
            sync.dma_start(labt[:], lab[:, :]).then_inc(dmac, 16)
            for t in range(NT):
                if t >= 2:
                    sync.wait_ge(vt, t - 1)
                    sync.wait_ge(at, t - 1)
                sync.dma_start(
                    yb[t % 2][:], y[:, t * FDT:(t + 1) * FDT]
                ).then_inc(dmat[t % 2], 16)
            sync.wait_ge(vt, NT + 2)
            sync.wait_ge(at, NT + 1)
            sync.dma_start(res[:, :], rest[:]).then_inc(dmat0, 16)
            sync.wait_ge(dmat0, 16 * (NT // 2 + 1))

        @block.vector
        def _(vector):
            for t in range(NT):
                yt = yb[t % 2]
                base = t * 9
                vector.wait_ge(dmat[t % 2], 16 * (t // 2 + 1))
                for k in range(1, 5):
                    nc.vector.tensor_scalar(
                        out=scrV[:], in0=yt[:], scalar1=float(EDGES[k]),
                        scalar2=0.0, op0=A.is_gt, op1=A.add,
                        accum_out=rest[:, base + k - 1:base + k])
                if t < 2:   # stot for tiles 0,1 on DVE (engine balance)
                    nc.vector.tensor_scalar(
                        out=scrV[:], in0=yt[:], scalar1=0.0,
                        scalar2=0.0, op0=A.add, op1=A.add,
                        accum_out=rest[:, base + 8:base + 9])
                # f32 max: for positive floats the f32 order equals the int
                # bit order, and the f32 comparator is exact (the int path
                # would round 2^30-scale ints to 24-bit mantissa).
                yg = yt[:].rearrange("p (g c) -> p g c", c=C)
                nc.vector.tensor_reduce(
                    out=pmt[:, t * GT:(t + 1) * GT].bitcast(f32), in_=yg,
                    axis=mybir.AxisListType.X, op=A.max,
                ).then_inc(vt, 1)
            # finals
            vector.wait_ge(dmac, 32)
            nc.vector.tensor_scalar(out=am[:], in0=pmt[:], scalar1=15,
                                    scalar2=None, op0=A.bitwise_and)
            nc.vector.tensor_scalar(out=pmc[:], in0=pmt[:], scalar1=-16,
                                    scalar2=None,
                                    op0=A.bitwise_and).then_inc(vt, 1)  # pmc ready for ACT
            nc.vector.tensor_tensor(out=corr[:], in0=am[:], in1=labt[:],
                                    op=A.is_equal)
            # corr -> 0/-1 (exact small-int mult), then bitwise-and keeps the
            # whole chain exact (int add/sub/mult run in the f32 datapath and
            # would round 2^30-scale values).
            nc.vector.tensor_scalar(out=corr[:], in0=corr[:], scalar1=-1,
                                    scalar2=None, op0=A.mult)
            nc.vector.tensor_tensor(out=pc[:], in0=pmc[:], in1=corr[:],
                                    op=A.bitwise_and)
            pmf = pmc[:].bitcast(f32)
            pcf = pc[:].bitcast(f32)
            for i, k in enumerate(range(5, 10)):
                nc.vector.tensor_scalar(
                    out=scrF[:], in0=pmf, scalar1=float(EDGES[k]),
                    scalar2=0.0, op0=A.is_gt, op1=A.add,
                    accum_out=rest[:, FB + i:FB + i + 1])
            ecs = [EC0] + [float(EDGES[k]) for k in range(1, 10)]
            for i, e in enumerate(ecs):
                nc.vector.tensor_scalar(
                    out=scrF[:], in0=pcf, scalar1=e,
                    scalar2=0.0, op0=A.is_gt, op1=A.add,
                    accum_out=rest[:, FB + 10 + i:FB + 11 + i])
            nc.vector.tensor_copy(scrF[:, 0:1], rest[:, 0:1]).then_inc(vt, 1)

        @block.scalar
        def _(scalar):
            scalar.wait_ge(dmac, 32)
            for t in range(NT):
                yt = yb[t % 2]
                base = t * 9
                scalar.wait_ge(dmat[t % 2], 16 * (t // 2 + 1))
                for k in range(1, 5):
                    nc.scalar.activation(
                        out=scrA[:], in_=yt[:], func=AF.Relu,
                        bias=biast[:, k:k + 1], scale=1.0,
                        accum_out=rest[:, base + 3 + k:base + 4 + k])
                if t >= 2:  # stot for tiles 2,3 on ACT
                    nc.scalar.activation(
                        out=scrA[:], in_=yt[:], func=AF.Relu,
                        bias=biast[:, 0:1], scale=1.0,
                        accum_out=rest[:, base + 8:base + 9])
                nc.scalar.activation(
                    out=scrA[:, 0:1], in_=yt[:, 0:1], func=AF.Relu,
                    bias=biast[:, 0:1], scale=1.0).then_inc(at, 1)
            # finals: relu sums for edges 5..9 on the packed-max array
            scalar.wait_ge(vt, NT + 1)   # pmc written
            pmf = pmc[:].bitcast(f32)
            for i, k in enumerate(range(5, 10)):
                nc.scalar.activation(
                    out=scrA[:, 0:GP], in_=pmf, func=AF.Relu,
                    bias=biast[:, k:k + 1], scale=1.0,
                    accum_out=rest[:, FB + 5 + i:FB + 6 + i])
            nc.scalar.activation(
                out=scrA[:, 0:1], in_=pmf[:, 0:1], func=AF.Relu,
                bias=biast[:, 0:1], scale=1.0).then_inc(at, 1)
    return nc


def _get_nc():
    if "nc" not in _CACHE:
        _CACHE["nc"] = _build_nc()
    return _CACHE["nc"]


def _prepare_in_maps(probs, labels):
    probs = np.ascontiguousarray(np.asarray(probs, dtype=np.float32))
    labels = np.ascontiguousarray(np.asarray(labels)).astype(np.int32)
    ib = probs.view(np.int32)
    pat = (9 - np.arange(C, dtype=np.int32))
    Y = (ib & np.int32(-16)) | pat[None, :]
    lenc = (9 - labels).astype(np.int32)
    biasv = np.zeros((P, 16), np.float32)
    for k in range(1, 10):
        biasv[:, k] = -EDGES[k]
    in_maps = []
    for c in range(NCORES):
        ypad = np.empty((NPAD, C), np.int32)
        ypad[:NS_CORE] = Y[c * NS_CORE:(c + 1) * NS_CORE]
        ypad[NS_CORE:] = pat[None, :]   # pad rows: tiny denormals, max decodes to 0.0
        lpad = np.empty((NPAD,), np.int32)
        lpad[:NS_CORE] = lenc[c * NS_CORE:(c + 1) * NS_CORE]
        lpad[NS_CORE:] = 9
        in_maps.append({
            "y": np.ascontiguousarray(ypad.reshape(P, GP * C).view(np.float32)),
            "lab": np.ascontiguousarray(lpad.reshape(P, GP)),
            "bias": biasv,
        })
    return in_maps


def _postprocess(res_sum):
    """res_sum: [RES_SLOTS] f64 summed over cores and partitions."""
    e = EDGES.astype(np.float64)
    G = np.zeros(11)
    R = np.zeros(11)
    G[0] = float(N) * C
    stot = 0.0
    for t in range(NT):
        for k in range(1, 5):
            G[k] += res_sum[t * 9 + k - 1]
            R[k] += res_sum[t * 9 + 3 + k]
        stot += res_sum[t * 9 + 8]
    for i, k in enumerate(range(5, 10)):
        G[k] = res_sum[FB + i]
        R[k] = res_sum[FB + 5 + i]
    gc = np.zeros(11)
    gc[:10] = res_sum[FB + 10:FB + 20]
    S = np.zeros(11)
    S[0] = stot
    for k in range(1, 10):
        S[k] = R[k] + e[k] * G[k]
    counts = G[:10] - G[1:11]
    confsum = S[:10] - S[1:11]
    corrects = gc[:10] - gc[1:11]
    acc = corrects / counts
    conf = confsum / counts
    total = counts.sum()
    ece = (np.abs(conf - acc) * counts).sum() / total
    centers = (EDGES[1:] - np.float32(0.1) / np.float32(2)).astype(np.float32)
    return (np.float32(ece), centers, acc.astype(np.float32))


def kernel(probs, labels):
    from concourse import bass_utils
    nc = _get_nc()
    in_maps = _prepare_in_maps(probs, labels)
    out = bass_utils.run_bass_kernel_spmd(nc, in_maps, core_ids=list(range(NCORES)))
    res = np.stack([np.asarray(r["res"], dtype=np.float64) for r in out.results])
    res_sum = res.sum(axis=(0, 1))
    return _postprocess(res_sum)


# revision 11
# speedup vs baseline: 3.5477x; 3.5477x over previous
"""ECE-loss kernel for Trainium2 (8 NeuronCores, raw Bass).

Strategy (validated against f64 ground truth on the reference dataset):
  - Histogram over 10 confidence bins from cumulative stats:
      G[k] = #{p > e_k},  R[k] = sum relu(p - e_k)   for edges e_1..e_9
    then counts[b] = G[b]-G[b+1], conf_sum[b] = S[b]-S[b+1] with
    S[k] = R[k] + e_k*G[k].  Only edges 0.1..0.4 need the full data:
    any p > 0.5 is necessarily the row max, so edges 0.5..0.9 are
    computed on the 10x smaller per-row max array.
  - Correctness (argmax == label) uses a host-side bit-pack: for positive
    floats the int32 view is order-preserving, so we send
      y = (bitcast_i32(p) & ~15) | (9 - class)
    and a single segmented int-max per row yields both the (truncated)
    max prob and its argmax; ties break toward the smallest class index,
    matching jnp.argmax.
  - Device reduces everything to per-partition scalars via
    tensor_scalar/activation accum_out (DVE: mask counts; ACT: relu sums);
    host does the final 30-scalar math in f64.

Data parallel over N across the 8 cores; per-core output is [128, 64] f32.
"""

import numpy as np

NCORES = 8
N = 2_000_000
C = 10
NS_CORE = N // NCORES            # 250_000 samples per core
P = 128
GP = 1960                        # samples per partition (padded; 128*1960 = 250_880)
NPAD = P * GP
NT = 4                           # data tiles
GT = GP // NT                    # samples per partition per tile
FDT = GT * C                     # free-dim elements per tile
RES_SLOTS = 64

EDGES = np.arange(0.0, 1.1, 0.1).astype(np.float32)   # same 11 edges as reference
EC0 = 0.05                       # below any real row-max (>=0.1), above pad zeros

# res slot map: per tile t: [t*9 .. t*9+8] = cnt_e1..4, relu_e1..4, stot_t
FB = NT * 9                      # finals base = 36
# finals: FB+0..4 cnt_e5..9 | FB+5..9 relu_e5..9 | FB+10..19 gcorr edges [0.05,e1..e9]

_CACHE = {}


def _build_nc(repeat=1):
    from contextlib import ExitStack
    import concourse.bass as bass
    import concourse.mybir as mybir

    A = mybir.AluOpType
    AF = mybir.ActivationFunctionType
    f32 = mybir.dt.float32
    i32 = mybir.dt.int32

    nc = bass.Bass("TRN2", target_bir_lowering=False, debug=False,
                   num_devices=NCORES)
    y = nc.dram_tensor("y", [P, GP * C], f32, kind="ExternalInput")
    lab = nc.dram_tensor("lab", [P, GP], i32, kind="ExternalInput")
    bias = nc.dram_tensor("bias", [P, 16], f32, kind="ExternalInput")
    res = nc.dram_tensor("res", [P, RES_SLOTS], f32, kind="ExternalOutput")

    with ExitStack() as st:
        yb = [st.enter_context(nc.sbuf_tensor(f"y{i}", [P, FDT], f32)) for i in range(2)]
        labt = st.enter_context(nc.sbuf_tensor("labt", [P, GP], i32))
        biast = st.enter_context(nc.sbuf_tensor("biast", [P, 16], f32))
        rest = st.enter_context(nc.sbuf_tensor("rest", [P, RES_SLOTS], f32))
        pmt = st.enter_context(nc.sbuf_tensor("pmt", [P, GP], i32))
        am = st.enter_context(nc.sbuf_tensor("am", [P, GP], i32))
        pmc = st.enter_context(nc.sbuf_tensor("pmc", [P, GP], i32))
        corr = st.enter_context(nc.sbuf_tensor("corr", [P, GP], i32))
        pc = st.enter_context(nc.sbuf_tensor("pc", [P, GP], i32))
        scrV = st.enter_context(nc.sbuf_tensor("scrV", [P, FDT], f32))
        scrA = st.enter_context(nc.sbuf_tensor("scrA", [P, FDT], f32))
        scrF = st.enter_context(nc.sbuf_tensor("scrF", [P, GP], f32))
        dmac = st.enter_context(nc.semaphore("dmac"))
        dmat0 = st.enter_context(nc.semaphore("dmat0"))
        dmat1 = st.enter_context(nc.semaphore("dmat1"))
        vt = st.enter_context(nc.semaphore("vt"))
        at = st.enter_context(nc.semaphore("at"))
        block = st.enter_context(nc.Block())

        dmat = [dmat0, dmat1]

        VT_PER = NT + 2          # DVE incs per iteration: NT tiles + pmc + end
        AT_PER = NT + 1          # ACT incs per iteration

        def vt_after(gt):        # vt value once global tile gt's DVE work is done
            return (gt // NT) * VT_PER + (gt % NT) + 1

        def at_after(gt):
            return (gt // NT) * AT_PER + (gt % NT) + 1

        @block.sync
        def _(sync):
            sync.dma_start(biast[:], bias[:, :]).then_inc(dmac, 16)
            sync.dma_start(labt[:], lab[:, :]).then_inc(dmac, 16)
            for gt in range(repeat * NT):
                t = gt % NT
                if gt >= 2:
                    sync.wait_ge(vt, vt_after(gt - 2))
                    sync.wait_ge(at, at_after(gt - 2))
                sync.dma_start(
                    yb[gt % 2][:], y[:, t * FDT:(t + 1) * FDT]
                ).then_inc(dmat[gt % 2], 16)
            sync.wait_ge(vt, VT_PER * repeat)
            sync.wait_ge(at, AT_PER * repeat)
            sync.dma_start(res[:, :], rest[:]).then_inc(dmat0, 16)
            sync.wait_ge(dmat0, 16 * ((repeat * NT + 1) // 2 + 1))

        @block.vector
        def _(vector):
          for r in range(repeat):
            for t in range(NT):
                gt = r * NT + t
                yt = yb[gt % 2]
                base = t * 9
                vector.wait_ge(dmat[gt % 2], 16 * (gt // 2 + 1))
                for k in range(1, 5):
                    nc.vector.tensor_scalar(
                        out=scrV[:], in0=yt[:], scalar1=float(EDGES[k]),
                        scalar2=0.0, op0=A.is_gt, op1=A.add,
                        accum_out=rest[:, base + k - 1:base + k])
                if t < 2:   # stot for tiles 0,1 on DVE (engine balance)
                    nc.vector.tensor_scalar(
                        out=scrV[:], in0=yt[:], scalar1=0.0,
                        scalar2=0.0, op0=A.add, op1=A.add,
                        accum_out=rest[:, base + 8:base + 9])
                # f32 max: for positive floats the f32 order equals the int
                # bit order, and the f32 comparator is exact (the int path
                # would round 2^30-scale ints to 24-bit mantissa).
                yg = yt[:].rearrange("p (g c) -> p g c", c=C)
                nc.vector.tensor_reduce(
                    out=pmt[:, t * GT:(t + 1) * GT].bitcast(f32), in_=yg,
                    axis=mybir.AxisListType.X, op=A.max,
                ).then_inc(vt, 1)
            # finals
            vector.wait_ge(dmac, 32)
            nc.vector.tensor_scalar(out=am[:], in0=pmt[:], scalar1=15,
                                    scalar2=None, op0=A.bitwise_and)
            nc.vector.tensor_scalar(out=pmc[:], in0=pmt[:], scalar1=-16,
                                    scalar2=None,
                                    op0=A.bitwise_and).then_inc(vt, 1)  # pmc ready for ACT
            nc.vector.tensor_tensor(out=corr[:], in0=am[:], in1=labt[:],
                                    op=A.is_equal)
            # corr -> 0/-1 (exact small-int mult), then bitwise-and keeps the
            # whole chain exact (int add/sub/mult run in the f32 datapath and
            # would round 2^30-scale values).
            nc.vector.tensor_scalar(out=corr[:], in0=corr[:], scalar1=-1,
                                    scalar2=None, op0=A.mult)
            nc.vector.tensor_tensor(out=pc[:], in0=pmc[:], in1=corr[:],
                                    op=A.bitwise_and)
            pmf = pmc[:].bitcast(f32)
            pcf = pc[:].bitcast(f32)
            for i, k in enumerate(range(5, 10)):
                nc.vector.tensor_scalar(
                    out=scrF[:], in0=pmf, scalar1=float(EDGES[k]),
                    scalar2=0.0, op0=A.is_gt, op1=A.add,
                    accum_out=rest[:, FB + i:FB + i + 1])
            ecs = [EC0] + [float(EDGES[k]) for k in range(1, 10)]
            for i, e in enumerate(ecs):
                nc.vector.tensor_scalar(
                    out=scrF[:], in0=pcf, scalar1=e,
                    scalar2=0.0, op0=A.is_gt, op1=A.add,
                    accum_out=rest[:, FB + 10 + i:FB + 11 + i])
            nc.vector.tensor_copy(scrF[:, 0:1], rest[:, 0:1]).then_inc(vt, 1)

        @block.scalar
        def _(scalar):
          scalar.wait_ge(dmac, 32)
          for r in range(repeat):
            for t in range(NT):
                gt = r * NT + t
                yt = yb[gt % 2]
                base = t * 9
                scalar.wait_ge(dmat[gt % 2], 16 * (gt // 2 + 1))
                for k in range(1, 5):
                    nc.scalar.activation(
                        out=scrA[:], in_=yt[:], func=AF.Relu,
                        bias=biast[:, k:k + 1], scale=1.0,
                        accum_out=rest[:, base + 3 + k:base + 4 + k])
                if t >= 2:  # stot for tiles 2,3 on ACT
                    nc.scalar.activation(
                        out=scrA[:], in_=yt[:], func=AF.Relu,
                        bias=biast[:, 0:1], scale=1.0,
                        accum_out=rest[:, base + 8:base + 9])
                nc.scalar.activation(
                    out=scrA[:, 0:1], in_=yt[:, 0:1], func=AF.Relu,
                    bias=biast[:, 0:1], scale=1.0).then_inc(at, 1)
            # finals: relu sums for edges 5..9 on the packed-max array
            # (in repeat>1 bench mode this races with the next iteration's
            # pmc rewrite — timing-only mode, results unused)
            scalar.wait_ge(vt, r * VT_PER + NT + 1)   # pmc written
            pmf = pmc[:].bitcast(f32)
            for i, k in enumerate(range(5, 10)):
                nc.scalar.activation(
                    out=scrA[:, 0:GP], in_=pmf, func=AF.Relu,
                    bias=biast[:, k:k + 1], scale=1.0,
                    accum_out=rest[:, FB + 5 + i:FB + 6 + i])
            nc.scalar.activation(
                out=scrA[:, 0:1], in_=pmf[:, 0:1], func=AF.Relu,
                bias=biast[:, 0:1], scale=1.0).then_inc(at, 1)
    return nc


def _get_nc():
    if "nc" not in _CACHE:
        _CACHE["nc"] = _build_nc()
    return _CACHE["nc"]


def _prepare_in_maps(probs, labels):
    probs = np.ascontiguousarray(np.asarray(probs, dtype=np.float32))
    labels = np.ascontiguousarray(np.asarray(labels)).astype(np.int32)
    ib = probs.view(np.int32)
    pat = (9 - np.arange(C, dtype=np.int32))
    Y = (ib & np.int32(-16)) | pat[None, :]
    lenc = (9 - labels).astype(np.int32)
    biasv = np.zeros((P, 16), np.float32)
    for k in range(1, 10):
        biasv[:, k] = -EDGES[k]
    in_maps = []
    for c in range(NCORES):
        ypad = np.empty((NPAD, C), np.int32)
        ypad[:NS_CORE] = Y[c * NS_CORE:(c + 1) * NS_CORE]
        ypad[NS_CORE:] = pat[None, :]   # pad rows: tiny denormals, max decodes to 0.0
        lpad = np.empty((NPAD,), np.int32)
        lpad[:NS_CORE] = lenc[c * NS_CORE:(c + 1) * NS_CORE]
        lpad[NS_CORE:] = 9
        in_maps.append({
            "y": np.ascontiguousarray(ypad.reshape(P, GP * C).view(np.float32)),
            "lab": np.ascontiguousarray(lpad.reshape(P, GP)),
            "bias": biasv,
        })
    return in_maps


def _postprocess(res_sum):
    """res_sum: [RES_SLOTS] f64 summed over cores and partitions."""
    e = EDGES.astype(np.float64)
    G = np.zeros(11)
    R = np.zeros(11)
    G[0] = float(N) * C
    stot = 0.0
    for t in range(NT):
        for k in range(1, 5):
            G[k] += res_sum[t * 9 + k - 1]
            R[k] += res_sum[t * 9 + 3 + k]
        stot += res_sum[t * 9 + 8]
    for i, k in enumerate(range(5, 10)):
        G[k] = res_sum[FB + i]
        R[k] = res_sum[FB + 5 + i]
    gc = np.zeros(11)
    gc[:10] = res_sum[FB + 10:FB + 20]
    S = np.zeros(11)
    S[0] = stot
    for k in range(1, 10):
        S[k] = R[k] + e[k] * G[k]
    counts = G[:10] - G[1:11]
    confsum = S[:10] - S[1:11]
    corrects = gc[:10] - gc[1:11]
    acc = corrects / counts
    conf = confsum / counts
    total = counts.sum()
    ece = (np.abs(conf - acc) * counts).sum() / total
    centers = (EDGES[1:] - np.float32(0.1) / np.float32(2)).astype(np.float32)
    return (np.float32(ece), centers, acc.astype(np.float32))


def kernel(probs, labels):
    from concourse import bass_utils
    nc = _get_nc()
    in_maps = _prepare_in_maps(probs, labels)
    out = bass_utils.run_bass_kernel_spmd(nc, in_maps, core_ids=list(range(NCORES)))
    res = np.stack([np.asarray(r["res"], dtype=np.float64) for r in out.results])
    res_sum = res.sum(axis=(0, 1))
    return _postprocess(res_sum)
